# revision 2
# baseline (speedup 1.0000x reference)
"""GCNEncoder Bass kernel for 8 TRN2 NeuronCores.

Algorithm (exact refactoring of the reference):
  ET3c[v]  = embed[v] @ (W1 @ Wn).T + (W1 @ bn)        # per-token fused table
  deg[n]   = 1 + |{e : dst_e = n}|                      # on-device one-hot matmuls
  dinv     = 1/sqrt(deg)
  m1[n]    = ET3c[tok[n]] * dinv[n]                     # prescaled layer-1 messages
  x1[n]    = relu(dinv[n] * sum_{e->n} m1[src_e] + b1)
  g2[n]    = (x1[n] @ W2.T) * dinv[n]                   # prescaled layer-2 messages
  out[n]   = dinv[n] * sum_{e->n} g2[src_e] + b2

Sharding: dst nodes split into 8 contiguous shards (one per core). Edges
(+self-loops) routed to the core owning dst. Aggregation via per-128-node-block
one-hot matmuls in PSUM, with edges grouped by src "wave" (32768-node ranges)
so gathers use int16 dma_gather from per-wave tables. Cross-core traffic:
per-wave AllGathers of the prescaled message tables (m1, g2).
"""

import sys
import numpy as np

sys.path.insert(0, "/opt/trn_rl_repo")

import concourse.bass as bass
import concourse.bacc as bacc
import concourse.mybir as mybir
import concourse.tile as tile
from concourse.bass_utils import run_bass_kernel_spmd
from concourse.masks import make_identity

P = 128
CC = 16           # chunks per dma_gather call (<= 2048 idx)
NQ = 4            # SWDGE queues

F32 = mybir.dt.float32
BF16 = mybir.dt.bfloat16
I16 = mybir.dt.int16

# message dtype ("bf16" or "f32")
MSG_DTYPE = "bf16"

PROFILE = False           # set True by test harness to get exec_time_ns
LAST_EXEC_NS = [None]


def full_cfg():
    return dict(N=131072, E=524288, V=32000, DIN=256, D=128, NCORES=8, B=64)


def derive_cfg(cfg):
    c = dict(cfg)
    N, NCORES = c["N"], c["NCORES"]
    c["NSH"] = NSH = N // NCORES          # nodes per core
    c["NBLK"] = NSH // P                  # dst blocks per core
    c["NG"] = NG = max(1, N // 32768)     # src waves / gather groups
    assert N % NG == 0 and NSH % NG == 0
    c["WV"] = NSH // NG                   # wave rows per core shard
    c["WROWS"] = NCORES * c["WV"]         # rows per wave table
    assert c["WROWS"] <= 32768
    assert c["V"] <= 32768 and c["V"] % P == 0
    assert c["D"] == 128 and c["DIN"] % P == 0
    c["BW"] = c["NBLK"] // NG             # blocks per wave (for G2/out order)
    assert c["NBLK"] % NG == 0
    return c


# ---------------------------------------------------------------- host prep

def pack_idx(flat):
    """Pack flat int idx list (len % 128 == 0) into dma_gather SBUF layout."""
    a = np.asarray(flat, dtype=np.int16)
    return np.tile(a.reshape(-1, 16).T, (8, 1))   # [128, len/16]


def host_prep(cfg, node_tokens, edge_index):
    c = cfg
    N, NCORES, NSH, NBLK, NG, WV = c["N"], c["NCORES"], c["NSH"], c["NBLK"], c["NG"], c["WV"]
    tok = np.asarray(node_tokens).astype(np.int64).ravel()
    src = np.asarray(edge_index[0]).astype(np.int64).ravel()
    dst = np.asarray(edge_index[1]).astype(np.int64).ravel()
    loops = np.arange(N, dtype=np.int64)
    src = np.concatenate([src, loops])
    dst = np.concatenate([dst, loops])

    core = dst // NSH
    blk = (dst % NSH) // P
    dloc = dst % P
    wv = (src % NSH) // WV                          # wave of src
    widx = (src // NSH) * WV + (src % WV)           # row within wave table

    key = (core * NG + wv) * NBLK + blk
    nseg = NCORES * NG * NBLK
    counts = np.bincount(key, minlength=nseg)
    S = int(np.ceil(counts.max() / P))              # chunks per segment (uniform)

    nchunks = NG * NBLK * S                         # per core per layer
    nslots = nchunks * P

    order = np.argsort(key, kind="stable")
    skey = key[order]
    first = np.zeros(nseg, dtype=np.int64)
    first[1:] = np.cumsum(counts)[:-1]
    pos = np.arange(len(order)) - first[skey]
    seg_in_core = skey % (NG * NBLK)
    slot = seg_in_core * (S * P) + pos
    score = skey // (NG * NBLK)

    gsrc = np.zeros((NCORES, nslots), dtype=np.int64)
    dstloc = np.full((NCORES, nslots), -1.0, dtype=np.float32)
    gsrc[score, slot] = widx[order]
    dstloc[score, slot] = dloc[order]

    # gather-call plan (shared across cores): per group, chunks in calls of CC
    calls = []
    col_off = 0
    for g in range(NG):
        base = g * NBLK * S
        k = 0
        while k < NBLK * S:
            nch = min(CC, NBLK * S - k)
            calls.append((g, base + k, nch, col_off))
            col_off += nch * P // 16
            k += nch
    idx_cols = col_off

    edge_idx = np.zeros((NCORES, P, idx_cols), dtype=np.int16)
    for ci in range(NCORES):
        for (g, ck, nch, off) in calls:
            flat = gsrc[ci, ck * P:(ck + nch) * P]
            edge_idx[ci, :, off:off + nch * P // 16] = pack_idx(flat)

    dstloc_cols = dstloc.reshape(NCORES, nchunks, P).transpose(0, 2, 1).copy()

    # m1 materialization call plan: own-shard tokens, per wave
    m1_calls = []     # (wave, node_chunk_start, nch, col_off)
    m1_off = 0
    for w in range(NG):
        k = 0
        while k < WV // P:
            nch = min(CC, WV // P - k)
            m1_calls.append((w, w * (WV // P) + k, nch, m1_off))
            m1_off += nch * P // 16
            k += nch
    tok_cols = m1_off
    tok_idx = np.zeros((NCORES, P, tok_cols), dtype=np.int16)
    for ci in range(NCORES):
        tt = tok[ci * NSH:(ci + 1) * NSH]
        for (w, ck0, nch, off) in m1_calls:
            ck_local = ck0 * P
            flat = tt[ck_local:ck_local + nch * P]
            tok_idx[ci, :, off:off + nch * P // 16] = pack_idx(flat)

    meta = dict(cfg)
    meta.update(S=S, nchunks=nchunks, calls=calls, idx_cols=idx_cols,
                m1_calls=m1_calls, tok_cols=tok_cols)
    percore = dict(edge_idx=edge_idx, dstloc=dstloc_cols, tok_idx=tok_idx)
    return meta, percore


# ---------------------------------------------------------------- device program

def build_program(meta):
    c = meta
    N, V, DIN, D = c["N"], c["V"], c["DIN"], c["D"]
    NCORES, NSH, NBLK, NG, WV, S = c["NCORES"], c["NSH"], c["NBLK"], c["NG"], c["WV"], c["S"]
    WROWS, BW = c["WROWS"], c["BW"]
    KH = DIN // P                      # DIN splits for matmul K
    VC = V // P                        # vocab chunks
    DT = BF16 if MSG_DTYPE == "bf16" else F32
    rg = [list(range(NCORES))]

    nc = bacc.Bacc("TRN2", target_bir_lowering=False, debug=False,
                   num_devices=NCORES, num_swdge_queues=NQ)

    # ---- kernel I/O
    embedT = nc.declare_dram_parameter("embedT", [DIN, V], F32, isOutput=False)
    Wn = nc.declare_dram_parameter("Wn", [D, DIN], F32, isOutput=False)
    W1T = nc.declare_dram_parameter("W1T", [D, D], F32, isOutput=False)
    W2T = nc.declare_dram_parameter("W2T", [D, D], F32, isOutput=False)
    bn = nc.declare_dram_parameter("bn", [D, 1], F32, isOutput=False)
    b1 = nc.declare_dram_parameter("b1", [1, D], F32, isOutput=False)
    b2 = nc.declare_dram_parameter("b2", [1, D], F32, isOutput=False)
    edge_idx_d = nc.declare_dram_parameter("edge_idx", [P, c["idx_cols"]], I16, isOutput=False)
    dstloc_d = nc.declare_dram_parameter("dstloc", [P, c["nchunks"]], F32, isOutput=False)
    tok_idx_d = nc.declare_dram_parameter("tok_idx", [P, c["tok_cols"]], I16, isOutput=False)
    out_d = nc.declare_dram_parameter("out_shard", [NSH, D], F32, isOutput=True)

    # ---- internal DRAM
    et3c = nc.dram_tensor("et3c", [V, D], DT)
    m1sh = [nc.dram_tensor(f"m1sh{w}", [WV, D], DT) for w in range(NG)]
    m1tab = [nc.dram_tensor(f"m1tab{w}", [WROWS, D], DT) for w in range(NG)]
    g2sh = [nc.dram_tensor(f"g2sh{w}", [WV, D], DT) for w in range(NG)]
    g2tab = [nc.dram_tensor(f"g2tab{w}", [WROWS, D], DT) for w in range(NG)]

    with tile.TileContext(nc) as tc:
        with (
            tc.tile_pool(name="const", bufs=1) as cst,
            tc.tile_pool(name="acc", bufs=1) as accp,
            tc.tile_pool(name="gat", bufs=4) as gatp,
            tc.tile_pool(name="oh", bufs=6) as ohp,
            tc.tile_pool(name="work", bufs=3) as wkp,
            tc.tile_pool(name="emb", bufs=4) as embp,
            tc.tile_pool(name="psA", bufs=4, space="PSUM") as psA,
            tc.tile_pool(name="psB", bufs=2, space="PSUM") as psB,
        ):
            # ---------- constants / small weights
            iota = cst.tile([P, P], F32, tag="iota")
            nc.gpsimd.iota(iota[:], pattern=[[1, P]], base=0, channel_multiplier=0,
                           allow_small_or_imprecise_dtypes=True)
            ones_dt = cst.tile([P, 1], DT, tag="ones")
            nc.vector.memset(ones_dt[:], 1.0)
            ident = cst.tile([P, P], F32, tag="ident")
            make_identity(nc, ident[:])

            wn_sb = cst.tile([D, DIN], F32, tag="wn")
            nc.sync.dma_start(out=wn_sb[:], in_=Wn[:])
            w1t_sb = cst.tile([D, D], F32, tag="w1t")
            nc.sync.dma_start(out=w1t_sb[:], in_=W1T[:])
            w2t_f = cst.tile([D, D], F32, tag="w2tf")
            nc.sync.dma_start(out=w2t_f[:], in_=W2T[:])
            w2t_sb = cst.tile([D, D], DT, tag="w2t")
            nc.vector.tensor_copy(out=w2t_sb[:], in_=w2t_f[:])
            bn_sb = cst.tile([D, 1], F32, tag="bn")
            nc.sync.dma_start(out=bn_sb[:], in_=bn[:])
            b1_row = cst.tile([1, D], F32, tag="b1r")
            nc.sync.dma_start(out=b1_row[:], in_=b1[:])
            b2_row = cst.tile([1, D], F32, tag="b2r")
            nc.sync.dma_start(out=b2_row[:], in_=b2[:])
            ones_row = cst.tile([1, P], F32, tag="onesr")
            nc.vector.memset(ones_row[:], 1.0)

            # index arrays
            eidx = cst.tile([P, c["idx_cols"]], I16, tag="eidx")
            nc.sync.dma_start(out=eidx[:], in_=edge_idx_d[:])
            dloc = cst.tile([P, c["nchunks"]], F32, tag="dloc")
            nc.sync.dma_start(out=dloc[:], in_=dstloc_d[:])
            tidx = cst.tile([P, c["tok_cols"]], I16, tag="tidx")
            nc.sync.dma_start(out=tidx[:], in_=tok_idx_d[:])

            # broadcast tiles for b1/b2 (rows replicated): K=1 matmul
            def bcast_row(row_sb, tag):
                ps = psB.tile([P, D], F32, space="PSUM", tag="misc")
                nc.tensor.matmul(out=ps[:], lhsT=ones_row[:], rhs=row_sb[:],
                                 start=True, stop=True)
                t = cst.tile([P, D], F32, tag=tag)
                nc.vector.tensor_copy(out=t[:], in_=ps[:])
                return t
            b1_tile = bcast_row(b1_row, "b1t")
            b2_tile = bcast_row(b2_row, "b2t")

            # M1 = W1 @ Wn  [D, DIN]
            m1ps = psB.tile([D, DIN], F32, space="PSUM", tag="misc")
            nc.tensor.matmul(out=m1ps[:], lhsT=w1t_sb[:], rhs=wn_sb[:],
                             start=True, stop=True)
            m1w = cst.tile([D, DIN], F32, tag="m1w")
            nc.vector.tensor_copy(out=m1w[:], in_=m1ps[:])
            # M1T halves (bf16) via PE transpose
            m1t_h = []
            for h in range(KH):
                tps = psB.tile([P, P], F32, space="PSUM", tag="misc")
                nc.tensor.transpose(out=tps[:], in_=m1w[:, h * P:(h + 1) * P],
                                    identity=ident[:])
                t = cst.tile([P, P], DT, tag=f"m1t{h}")
                nc.vector.tensor_copy(out=t[:], in_=tps[:])
                m1t_h.append(t)
            # c1 = W1 @ bn as row [1, D]
            c1ps = psB.tile([1, D], F32, space="PSUM", tag="misc")
            nc.tensor.matmul(out=c1ps[:], lhsT=bn_sb[:], rhs=w1t_sb[:],
                             start=True, stop=True)
            c1row = cst.tile([1, D], F32, tag="c1r")
            nc.vector.tensor_copy(out=c1row[:], in_=c1ps[:])
            c1_tile = bcast_row(c1row, "c1t")

            # ---------- ET3c table build: et3c[v] = embed[v] @ M1.T + c1
            for vc in range(VC):
                ps = psB.tile([P, D], F32, space="PSUM", tag="misc")
                for h in range(KH):
                    et = embp.tile([P, P], DT, tag="embt")
                    nc.gpsimd.dma_start(
                        out=et[:], in_=embedT[h * P:(h + 1) * P, vc * P:(vc + 1) * P])
                    nc.tensor.matmul(out=ps[:], lhsT=et[:], rhs=m1t_h[h][:],
                                     start=(h == 0), stop=(h == KH - 1))
                ot = embp.tile([P, D], DT, tag="etout")
                nc.vector.scalar_tensor_tensor(
                    out=ot[:], in0=ps[:], scalar=1.0, in1=c1_tile[:],
                    op0=mybir.AluOpType.mult, op1=mybir.AluOpType.add)
                nc.sync.dma_start(out=et3c[vc * P:(vc + 1) * P, :], in_=ot[:])

            # ---------- degree pass (one-hot column sums)
            degT = cst.tile([P, NBLK], F32, tag="degT")
            for g in range(NG):
                for b in range(NBLK):
                    dps = psB.tile([P, 1], F32, space="PSUM", tag="misc")
                    for s in range(S):
                        k = (g * NBLK + b) * S + s
                        oh = ohp.tile([P, P], DT, tag="oh")
                        nc.vector.tensor_scalar(
                            out=oh[:], in0=iota[:], scalar1=dloc[:, k:k + 1],
                            scalar2=None, op0=mybir.AluOpType.is_equal)
                        nc.tensor.matmul(out=dps[:], lhsT=oh[:], rhs=ones_dt[:],
                                         start=(s == 0), stop=(s == S - 1))
                    if g == 0:
                        nc.vector.tensor_copy(out=degT[:, b:b + 1], in_=dps[:])
                    else:
                        nc.vector.tensor_add(out=degT[:, b:b + 1],
                                             in0=degT[:, b:b + 1], in1=dps[:])
            sqd = cst.tile([P, NBLK], F32, tag="sqd")
            nc.scalar.sqrt(sqd[:], degT[:])
            dinvT = cst.tile([P, NBLK], F32, tag="dinvT")
            nc.vector.reciprocal(out=dinvT[:], in_=sqd[:])

            # ---------- m1 materialization + per-wave AllGather
            qrr = [0]
            def next_q():
                q = qrr[0]
                qrr[0] = (q + 1) % NQ
                return q

            mcalls_by_wave = {}
            for (w, ck0, nch, off) in c["m1_calls"]:
                mcalls_by_wave.setdefault(w, []).append((ck0, nch, off))
            for w in range(NG):
                for (ck0, nch, off) in mcalls_by_wave[w]:
                    nidx = nch * P
                    gt = gatp.tile([P, CC * D], DT, tag="gat")
                    g3 = gt[:, :nch * D].rearrange("p (c e) -> p c e", c=nch)
                    nc.gpsimd.dma_gather(
                        g3, et3c[:], tidx[:, off:off + nidx // 16], nidx, nidx, D,
                        queue_num=next_q(), single_packet=False)
                    m1t = gatp.tile([P, CC * D], DT, tag="m1o")
                    for ci in range(nch):
                        blk_i = ck0 + ci
                        nc.vector.tensor_scalar(
                            out=m1t[:, ci * D:(ci + 1) * D],
                            in0=gt[:, ci * D:(ci + 1) * D],
                            scalar1=dinvT[:, blk_i:blk_i + 1], scalar2=None,
                            op0=mybir.AluOpType.mult)
                    # rows (ck0-local within wave)*P .. of m1sh[w]
                    r0 = (ck0 - w * (WV // P)) * P
                    dst_ap = m1sh[w][r0:r0 + nch * P, :].rearrange(
                        "(C p) e -> p C e", p=P)
                    nc.sync.dma_start(out=dst_ap, in_=m1t[:, :nch * D].rearrange(
                        "p (C e) -> p C e", C=nch))
                nc.gpsimd.collective_compute(
                    "AllGather", mybir.AluOpType.bypass, replica_groups=rg,
                    ins=[m1sh[w].ap().opt()], outs=[m1tab[w].ap().opt()])

            # ---------- shared edge-pass loop
            def edge_pass(tabs, acc_tag, finalize):
                acc = accp.tile([P, NSH], F32, tag="acc")
                calls_by_g = {}
                for (g, ck, nch, off) in c["calls"]:
                    calls_by_g.setdefault(g, []).append((ck, nch, off))
                for g in range(NG):
                    # chunk index -> gathered tile/slice resolved per call
                    for (ck, nch, off) in calls_by_g[g]:
                        nidx = nch * P
                        gt = gatp.tile([P, CC * D], DT, tag="gat")
                        g3 = gt[:, :nch * D].rearrange("p (c e) -> p c e", c=nch)
                        nc.gpsimd.dma_gather(
                            g3, tabs[g][:], eidx[:, off:off + nidx // 16],
                            nidx, nidx, D, queue_num=next_q(), single_packet=False)
                        for ci in range(nch):
                            k = ck + ci
                            r = k - g * NBLK * S
                            b, s = r // S, r % S
                            oh = ohp.tile([P, P], DT, tag="oh")
                            nc.vector.tensor_scalar(
                                out=oh[:], in0=iota[:], scalar1=dloc[:, k:k + 1],
                                scalar2=None, op0=mybir.AluOpType.is_equal)
                            if s == 0:
                                aps = psA.tile([P, D], F32, space="PSUM", tag="agg")
                                edge_pass.cur = aps
                            aps = edge_pass.cur
                            nc.tensor.matmul(out=aps[:], lhsT=oh[:],
                                             rhs=gt[:, ci * D:(ci + 1) * D],
                                             start=(s == 0), stop=(s == S - 1))
                            if s == S - 1:
                                sl = acc[:, b * D:(b + 1) * D]
                                if g == 0:
                                    nc.vector.tensor_copy(out=sl, in_=aps[:])
                                else:
                                    nc.vector.tensor_add(out=sl, in0=sl, in1=aps[:])
                                if g == NG - 1:
                                    finalize(b, sl)
                return acc

            # ---------- layer 1
            def fin1(b, sl):
                t = wkp.tile([P, D], F32, tag="fin")
                nc.vector.scalar_tensor_tensor(
                    out=t[:], in0=sl, scalar=dinvT[:, b:b + 1], in1=b1_tile[:],
                    op0=mybir.AluOpType.mult, op1=mybir.AluOpType.add)
                x1 = wkp.tile([P, D], F32, tag="x1")
                nc.scalar.activation(x1[:], t[:], mybir.ActivationFunctionType.Relu)
                tp = psB.tile([P, P], F32, space="PSUM", tag="misc")
                nc.tensor.transpose(out=tp[:], in_=x1[:], identity=ident[:])
                x1t = wkp.tile([P, P], DT, tag="x1t")
                nc.vector.tensor_copy(out=x1t[:], in_=tp[:])
                hps = psB.tile([P, D], F32, space="PSUM", tag="misc")
                nc.tensor.matmul(out=hps[:], lhsT=x1t[:], rhs=w2t_sb[:],
                                 start=True, stop=True)
                g2t = wkp.tile([P, D], DT, tag="g2t")
                nc.scalar.activation(g2t[:], hps[:],
                                     mybir.ActivationFunctionType.Copy,
                                     scale=dinvT[:, b:b + 1])
                w = b // BW
                r0 = (b - w * BW) * P
                nc.sync.dma_start(out=g2sh[w][r0:r0 + P, :], in_=g2t[:])
                if (b + 1) % BW == 0:
                    nc.gpsimd.collective_compute(
                        "AllGather", mybir.AluOpType.bypass, replica_groups=rg,
                        ins=[g2sh[w].ap().opt()], outs=[g2tab[w].ap().opt()])

            edge_pass(m1tab, "acc1", fin1)

            # ---------- layer 2
            def fin2(b, sl):
                t = wkp.tile([P, D], F32, tag="fin")
                nc.vector.scalar_tensor_tensor(
                    out=t[:], in0=sl, scalar=dinvT[:, b:b + 1], in1=b2_tile[:],
                    op0=mybir.AluOpType.mult, op1=mybir.AluOpType.add)
                nc.sync.dma_start(out=out_d[b * P:(b + 1) * P, :], in_=t[:])

            edge_pass(g2tab, "acc2", fin2)

    nc.compile()
    return nc


# ---------------------------------------------------------------- runner

_CACHE = {}


def run(cfg, node_tokens, edge_index, embed_table, W_node_w, W_node_b,
        conv1_w, conv1_b, conv2_w, conv2_b, trace=False):
    cfg = derive_cfg(cfg)
    meta, percore = host_prep(cfg, node_tokens, edge_index)
    key = (meta["N"], meta["E"], meta["V"], meta["S"], MSG_DTYPE)
    if key not in _CACHE:
        _CACHE[key] = build_program(meta)
    nc = _CACHE[key]

    D, DIN = cfg["D"], cfg["DIN"]
    embedT = np.ascontiguousarray(np.asarray(embed_table, dtype=np.float32).T)
    shared = dict(
        embedT=embedT,
        Wn=np.asarray(W_node_w, dtype=np.float32),
        W1T=np.ascontiguousarray(np.asarray(conv1_w, dtype=np.float32).T),
        W2T=np.ascontiguousarray(np.asarray(conv2_w, dtype=np.float32).T),
        bn=np.asarray(W_node_b, dtype=np.float32).reshape(D, 1),
        b1=np.asarray(conv1_b, dtype=np.float32).reshape(1, D),
        b2=np.asarray(conv2_b, dtype=np.float32).reshape(1, D),
    )
    in_maps = []
    for ci in range(cfg["NCORES"]):
        m = dict(shared)
        m["edge_idx"] = percore["edge_idx"][ci]
        m["dstloc"] = percore["dstloc"][ci]
        m["tok_idx"] = percore["tok_idx"][ci]
        in_maps.append(m)

    if trace:
        try:
            sys.path.insert(0, "/root/problem/hooks")
            import ntff_hook
            ntff_hook.install()
        except Exception:
            pass
    res = run_bass_kernel_spmd(nc, in_maps, list(range(cfg["NCORES"])), trace=trace)
    LAST_EXEC_NS[0] = res.exec_time_ns
    x = np.concatenate([res.results[ci]["out_shard"] for ci in range(cfg["NCORES"])],
                       axis=0)
    return x


def kernel(node_tokens, edge_index, embed_table, W_node_w, W_node_b,
           conv1_w, conv1_b, conv2_w, conv2_b):
    cfg = full_cfg()
    x = run(cfg, node_tokens, edge_index, embed_table, W_node_w, W_node_b,
            conv1_w, conv1_b, conv2_w, conv2_b, trace=PROFILE)
    B = cfg["B"]
    N, D = cfg["N"], cfg["D"]
    n = N // B
    tokens = np.asarray(node_tokens)
    output = x.reshape(B, n, D)
    labels = tokens.reshape(B, n)
    labels_mask = np.ones((B, n), dtype=bool)
    idt = np.int64 if tokens.dtype == np.int64 else np.int32
    label_node_ids = np.broadcast_to(np.arange(n, dtype=idt), (B, n)).copy()
    return output, labels, labels_mask, label_node_ids
